# revision 29
# baseline (speedup 1.0000x reference)
"""ArcFace combined-margin loss kernel for 8 TRN2 NeuronCores.

Strategy
--------
reference: cos = (f @ w.T) / (|f||w|); phi = arcface(cos);
outputs = s*(labels*phi + (1-labels)*cos); loss = mean over rows of
-(sum of log_softmax(outputs) at lab_pinds, masked) / L^2.

labels is the multi-hot of (lab_pinds, lengths), so outputs differs from
s*cos only at <=8 entries/row.  Device work:
  1. C-sharded (2500 classes/core, zero-padded to 2560) dense part: each
     core computes, for all 2048 rows, partial sums
     sexp[b] = sum_c exp(30*cos[b,c] - 30) over its class shard.
     The dots run as fp8(e4m3) DoubleRow matmuls (K=256 per instruction)
     of unit-normalized w rows against raw f rows, both pre-scaled by 16
     (absorbed by the ACT exp scale 30/(256*|f_b|), which also folds the
     row norm of f).  Operand transposes (contraction dim -> partitions)
     are HWDGE DMA-transposes of uint16-paired fp8 staged in DRAM — the
     16-bit transpose granularity lands adjacent-d pairs on one
     partition, exactly the [K,2,*] layout DoubleRow contracts.
  2. B-sharded (256 rows/core) positive part: indirect-DMA gather of the
     2048 w rows addressed by lab_pinds, full-fp32 dots on DVE (the
     positives feed the loss directly, so they stay in fp32).
  3. Per-row norm reciprocals (30/|f_b| and 1/|w_c|) as side outputs.
Host (numpy, float64) combines the tiny per-core partials: assembles
cos at positives, applies the arcface margin, corrects the denominator
(exp(30*phi)-exp(30*cos) at positives, minus the zero-pad classes),
logsumexp, masked ragged CE, mean.  No collectives: the only cross-core
reduction is over [2048]-sized vectors, done on host while unsharding.
"""

import math
import sys

import numpy as np

for _p in ("/opt/trn_rl_repo",):
    if _p not in sys.path:
        sys.path.append(_p)

import concourse.bass as bass
import concourse.bacc as bacc
import concourse.mybir as mybir
import concourse.tile as tile
from concourse.bass_utils import run_bass_kernel_spmd
from contextlib import ExitStack

B, C, D, LMAX = 2048, 20000, 512, 8
NCORES = 8
CSH = C // NCORES          # 2500 real classes per core
CSHP = 2560                # padded to 5*512
BSH = B // NCORES          # 256 rows per core (positives shard)
NBLK = B // 128            # 16 row blocks
NW = 512                   # matmul N-chunk width (one PSUM bank)
NCH = CSHP // NW           # 5 chunks per core
CT = CSHP // 128           # 20 class tiles (all full)
S = 30.0
M_MARGIN = 0.5
F32 = mybir.dt.float32
BF16 = mybir.dt.bfloat16

_GRAPH = None


def build_graph():
    nc = bacc.Bacc()
    f_ext = nc.declare_dram_parameter("f", [B, D], F32, isOutput=False)
    wsh_ext = nc.declare_dram_parameter("wsh", [CSHP, D], F32, isOutput=False)
    w_ext = nc.declare_dram_parameter("w", [C, D], F32, isOutput=False)
    fsh_ext = nc.declare_dram_parameter("fsh", [BSH, D], F32, isOutput=False)
    pidx_ext = nc.declare_dram_parameter("pidx", [128, 16], mybir.dt.int32, isOutput=False)
    sexp_ext = nc.declare_dram_parameter("sexp", [128, NBLK], F32, isOutput=True)
    pdot_ext = nc.declare_dram_parameter("pdot", [128, 16], F32, isOutput=True)
    rf_ext = nc.declare_dram_parameter("rf30", [128, NBLK], F32, isOutput=True)
    rw_ext = nc.declare_dram_parameter("rwrec", [128, CT], F32, isOutput=True)

    mult = mybir.AluOpType.mult
    AF = mybir.ActivationFunctionType
    HB = B // 2        # f rows per DRAM-bounce half
    HW = CSHP // 2     # w rows per half

    with ExitStack() as ctx:
        tc = ctx.enter_context(tile.TileContext(nc))
        const = ctx.enter_context(tc.tile_pool(name="const", bufs=1))
        resident = ctx.enter_context(tc.tile_pool(name="resident", bufs=1))
        fstage = ctx.enter_context(tc.tile_pool(name="fstage", bufs=3))
        wstage = ctx.enter_context(tc.tile_pool(name="wstage", bufs=1))
        w8p = ctx.enter_context(tc.tile_pool(name="w8p", bufs=3))
        scrp = ctx.enter_context(tc.tile_pool(name="scrp", bufs=2))
        esp = ctx.enter_context(tc.tile_pool(name="esp", bufs=2))
        dramp = ctx.enter_context(tc.tile_pool(name="dramp", bufs=1, space="DRAM"))
        pmm_pool = ctx.enter_context(tc.tile_pool(name="pmm", bufs=7, space="PSUM"))

        zbias = const.tile([128, 1], F32)
        nc.vector.memset(zbias[:], 0.0)
        nbias = const.tile([128, 1], F32)
        nc.vector.memset(nbias[:], -S)

        # DRAM staging for the bf16 operands, split in halves so the
        # DMA-transposes only wait on their half's stores.
        f8d = [dramp.tile([HB, D], BF16, name=f"f8d{h}") for h in range(2)]
        w8d = [dramp.tile([HW, D], BF16, name=f"w8d{h}") for h in range(2)]
        # transposed operands, one tile per 128-wide K chunk
        KC = D // 128
        fT = [resident.tile([128, B], BF16, name=f"fT_{k}") for k in range(KC)]
        wT = [resident.tile([128, CSHP], BF16, name=f"wT_{k}") for k in range(KC)]

        G = resident.tile([128, 16, D], F32)         # gathered positive w rows
        fsh_t = resident.tile([128, 2, D], F32)      # this core's f rows (raw)
        sexp_t = resident.tile([128, NBLK], F32)
        pdot_t = resident.tile([128, 16], F32)
        ss_f = resident.tile([128, NBLK], F32)
        tmp_f = resident.tile([128, NBLK], F32)
        rf30 = resident.tile([128, NBLK], F32)
        ss_w = resident.tile([128, CT], F32)
        tmp_w = resident.tile([128, CT], F32)
        rw_rec = resident.tile([128, CT], F32)
        pidx_t = resident.tile([128, 16], mybir.dt.int32)

        # ---- positives gather (SWDGE; result only needed at the end) ----
        nc.sync.dma_start(pidx_t[:], pidx_ext[:, :])
        nc.sync.dma_start(fsh_t[:, 0, :], fsh_ext[0:128, :])
        nc.sync.dma_start(fsh_t[:, 1, :], fsh_ext[128:256, :])
        for s in range(16):
            nc.gpsimd.indirect_dma_start(
                out=G[:, s, :],
                out_offset=None,
                in_=w_ext[:, :],
                in_offset=bass.IndirectOffsetOnAxis(ap=pidx_t[:, s : s + 1], axis=0),
            )

        # ---- interleaved w/f tile processing: norms + fp8 stage ----
        wtiles = {}

        def w_tile(ct):
            r0 = ct * 128
            wt = wstage.tile([128, D], F32, tag=f"w{ct % 12}", name=f"w{ct}")
            wtiles[ct] = wt
            nc.scalar.dma_start(wt[:, :], wsh_ext[r0 : r0 + 128, :])
            sc = scrp.tile([128, D], F32, tag="scr", name=f"scw{ct}")
            nc.vector.scalar_tensor_tensor(
                out=sc[:], in0=wt[:], scalar=1.0, in1=wt[:],
                op0=mult, op1=mult,
                accum_out=ss_w[:, ct : ct + 1],
            )

        def w_scale_store(ct):
            r0 = ct * 128
            w8 = w8p.tile([128, D], BF16, tag="w8", name=f"w8_{ct}")
            nc.vector.tensor_scalar_mul(
                w8[:, :], wtiles[ct][:, :], rw_rec[:, ct : ct + 1]
            )
            h, lr = divmod(r0, HW)
            nc.sync.dma_start(w8d[h][lr : lr + 128, :], w8[:, :])

        def w_norm_batch(c0, c1):
            # zero pad rows: keep sqrt/recip finite
            nc.vector.tensor_scalar_max(ss_w[:, c0:c1], ss_w[:, c0:c1], 1e-12)
            nc.scalar.activation(
                tmp_w[:, c0:c1], ss_w[:, c0:c1], AF.Sqrt,
                bias=zbias[:], scale=1.0,
            )
            nc.vector.reciprocal(rw_rec[:, c0:c1], tmp_w[:, c0:c1])

        def f_tile(m):
            ft = fstage.tile([128, D], F32, tag="f", name=f"f{m}")
            nc.scalar.dma_start(ft[:], f_ext[m * 128 : (m + 1) * 128, :])
            sc = scrp.tile([128, D], F32, tag="scr", name=f"scf{m}")
            nc.scalar.activation(
                sc[:], ft[:], AF.Square, bias=zbias[:], scale=1.0,
                accum_out=ss_f[:, m : m + 1],
            )
            f8 = w8p.tile([128, D], BF16, tag="f8", name=f"f8_{m}")
            nc.vector.tensor_copy(out=f8[:, :], in_=ft[:, :])
            h, lr = divmod(m * 128, HB)
            nc.sync.dma_start(f8d[h][lr : lr + 128, :], f8[:, :])

        QW = CSHP // 4   # 640 w rows per transpose quarter
        QF = B // 4      # 512 f rows per quarter

        def w_transposes(q):
            h, q0 = divmod(q * QW, HW)
            for k in range(KC):
                nc.sync.dma_start_transpose(
                    wT[k][:, q * QW : (q + 1) * QW],
                    w8d[h][q0 : q0 + QW, k * 128 : (k + 1) * 128],
                )

        def f_transposes(q):
            h, q0 = divmod(q * QF, HB)
            for k in range(KC):
                nc.sync.dma_start_transpose(
                    fT[k][:, q * QF : (q + 1) * QF],
                    f8d[h][q0 : q0 + QF, k * 128 : (k + 1) * 128],
                )

        WQ_T, FQ_T = CT // 4, NBLK // 4   # tiles per quarter (5 w, 4 f)
        for q in range(4):
            for i in range(q * WQ_T, (q + 1) * WQ_T):
                w_tile(i)
            w_norm_batch(q * WQ_T, (q + 1) * WQ_T)
            for i in range(q * WQ_T, (q + 1) * WQ_T):
                w_scale_store(i)
            w_transposes(q)
            for i in range(q * FQ_T, (q + 1) * FQ_T):
                f_tile(i)
            f_transposes(q)
        nc.sync.dma_start(rw_ext[:, :], rw_rec[:])

        # rf30 = 30 / |f|
        nc.scalar.activation(
            tmp_f[:], ss_f[:], AF.Sqrt, bias=zbias[:], scale=1.0 / (S * S)
        )
        nc.vector.reciprocal(rf30[:], tmp_f[:])
        nc.sync.dma_start(rf_ext[:, :], rf30[:])

        # ---- main loop: bf16 dots -> strip -> exp accumulate ----
        for m in range(NBLK):
            ps = [
                pmm_pool.tile([128, NW], F32, tag="mm", name=f"mm_{m}_{n}")
                for n in range(NCH)
            ]
            for k in range(KC):
                for n in range(NCH):
                    nc.tensor.matmul(
                        ps[n][:],
                        fT[k][:, m * 128 : (m + 1) * 128],
                        wT[k][:, n * NW : (n + 1) * NW],
                        start=(k == 0),
                        stop=(k == KC - 1),
                    )
            strip = esp.tile([128, NCH * NW], BF16, tag="es", name=f"st{m}")
            for n in range(NCH):
                nc.any.tensor_copy(
                    out=strip[:, n * NW : (n + 1) * NW], in_=ps[n][:]
                )
            edump = scrp.tile([128, NCH * NW], BF16, tag="edump", name=f"ed{m}")
            nc.scalar.activation(
                edump[:],
                strip[:],
                AF.Exp,
                bias=nbias[:], scale=rf30[:, m : m + 1],
                accum_out=sexp_t[:, m : m + 1],
            )
        nc.sync.dma_start(sexp_ext[:, :], sexp_t[:])

        # ---- positive dots: pdot[p, j+8h] = f[h*128+p] . G[p, j+8h] ----
        for h in range(2):
            for j in range(LMAX):
                s = j + LMAX * h
                sc = scrp.tile([128, D], F32, tag="scr", name=f"pd{s}")
                nc.vector.scalar_tensor_tensor(
                    out=sc[:], in0=G[:, s, :], scalar=1.0, in1=fsh_t[:, h, :],
                    op0=mult, op1=mult,
                    accum_out=pdot_t[:, s : s + 1],
                )
        nc.sync.dma_start(pdot_ext[:, :], pdot_t[:])

    nc.finalize()
    return nc


def _get_graph():
    global _GRAPH
    if _GRAPH is None:
        _GRAPH = build_graph()
    return _GRAPH


def make_in_maps(f, lab_word2vec, lab_pinds):
    f = np.ascontiguousarray(np.asarray(f, dtype=np.float32))
    w = np.ascontiguousarray(np.asarray(lab_word2vec, dtype=np.float32))
    pinds = np.asarray(lab_pinds, dtype=np.int64)
    in_maps = []
    for i in range(NCORES):
        # slot s = j + 8h at partition p  <-  lab_pinds[i*256 + h*128 + p, j]
        pidx = np.zeros((128, 16), dtype=np.int32)
        for h in range(2):
            for j in range(LMAX):
                pidx[:, j + LMAX * h] = pinds[
                    i * BSH + h * 128 : i * BSH + h * 128 + 128, j
                ]
        wsh = np.zeros((CSHP, D), dtype=np.float32)
        wsh[:CSH] = w[i * CSH : (i + 1) * CSH]
        in_maps.append(
            {
                "f": f,
                "wsh": wsh,
                "w": w,
                "fsh": np.ascontiguousarray(f[i * BSH : (i + 1) * BSH]),
                "pidx": pidx,
            }
        )
    return in_maps


def combine(outs, lab_pinds, lengths):
    """outs: list of 8 dicts with sexp/pdot/rf30/rwrec. Returns float32 loss."""
    pinds = np.asarray(lab_pinds, dtype=np.int64)
    lens = np.asarray(lengths, dtype=np.int64)

    # S_shift[b] = sum_c exp(30 cos - 30); subtract the 60 zero-pad classes
    # per core (cos=0 there -> exp(-30) each).
    s_shift = np.zeros(B, dtype=np.float64)
    for i in range(NCORES):
        se = outs[i]["sexp"].astype(np.float64)  # [128, NBLK]
        s_shift += se.T.reshape(B)  # b = m*128 + p
    s_shift -= NCORES * (CSHP - CSH) * math.exp(-S)

    rf = outs[0]["rf30"].astype(np.float64).T.reshape(B) / S  # 1/|f_b|

    rw = np.zeros(C, dtype=np.float64)
    for i in range(NCORES):
        rr = outs[i]["rwrec"].astype(np.float64)  # [128, CT]
        flat = rr.T.reshape(CSHP)[:CSH]  # c = ct*128 + p
        rw[i * CSH : (i + 1) * CSH] = flat

    # positive raw dots -> [B, LMAX]
    pdot = np.zeros((B, LMAX), dtype=np.float64)
    for i in range(NCORES):
        pd = outs[i]["pdot"].astype(np.float64)  # [128, 16]
        for h in range(2):
            for j in range(LMAX):
                pdot[i * BSH + h * 128 : i * BSH + h * 128 + 128, j] = pd[
                    :, j + LMAX * h
                ]

    cos = pdot * rf[:, None] * rw[pinds]  # [B, LMAX]
    cos_m, sin_m = math.cos(M_MARGIN), math.sin(M_MARGIN)
    th = math.cos(math.pi - M_MARGIN)
    mm = math.sin(math.pi - M_MARGIN) * M_MARGIN
    sine = np.sqrt(np.clip(1.0 - cos * cos, 0.0, 1.0))
    phi = cos * cos_m - sine * sin_m
    phi = np.where(cos > th, phi, cos - mm)

    mask = (np.arange(LMAX)[None, :] < lens[:, None]).astype(np.float64)
    corr = (mask * (np.exp(S * phi - S) - np.exp(S * cos - S))).sum(axis=1)
    z = S + np.log(s_shift + corr)  # logsumexp of outputs, [B]
    pos_sum = (mask * (S * phi)).sum(axis=1)
    L = lens.astype(np.float64)
    per_sample = (L * z - pos_sum) / (L * L)
    return np.float32(per_sample.mean())


def kernel(f, labels, lab_word2vec, lab_pinds, lengths):
    nc = _get_graph()
    in_maps = make_in_maps(f, lab_word2vec, lab_pinds)
    res = run_bass_kernel_spmd(nc, in_maps, core_ids=list(range(NCORES)))
    return combine(res.results, lab_pinds, lengths)


# revision 32
# speedup vs baseline: 1.4885x; 1.4885x over previous
"""ArcFace combined-margin loss kernel for 8 TRN2 NeuronCores.

Strategy
--------
reference: cos = (f @ w.T) / (|f||w|); phi = arcface(cos);
outputs = s*(labels*phi + (1-labels)*cos); loss = mean over rows of
-(sum of log_softmax(outputs) at lab_pinds, masked) / L^2.

labels is the multi-hot of (lab_pinds, lengths), so outputs differs from
s*cos only at <=8 entries/row.  Device work is therefore:
  1. C-sharded (2500 classes/core) dense part: each core computes, for all
     2048 rows, partial sums  sexp[b] = sum_c exp(30*cos[b,c] - 30)  over its
     class shard (bf16 matmul of unit-normalized w rows against raw f rows,
     transposed on the TensorEngine; ACT exp with per-partition scale
     30/|f_b| and free-dim accumulate, reading PSUM directly).
  2. B-sharded (256 rows/core) positive part: indirect-DMA gather of the
     2048 w rows addressed by lab_pinds, raw fp32 dots with f rows on DVE.
  3. Per-row norm reciprocals (30/|f_b| and 1/|w_c|) as side outputs.
Host (numpy, float64) combines the tiny per-core partials: assembles
cos at positives, applies the arcface margin, corrects the denominator
(exp(30*phi)-exp(30*cos) at positives), logsumexp, masked ragged CE, mean.
No collectives are needed (the only cross-core reduction is over [2048]
scalars, done on host during unsharding).
"""

import math
import sys

import numpy as np

for _p in ("/opt/trn_rl_repo",):
    if _p not in sys.path:
        sys.path.append(_p)

import concourse.bass as bass
import concourse.bacc as bacc
import concourse.mybir as mybir
import concourse.tile as tile
from concourse.bass_utils import run_bass_kernel_spmd
from concourse.masks import make_identity
from contextlib import ExitStack

B, C, D, LMAX = 2048, 20000, 512, 8
NCORES = 8
CSH = C // NCORES          # 2500 classes per core
BSH = B // NCORES          # 256 rows per core (positives shard)
NBLK = B // 128            # 16 row blocks
NW = 500                   # matmul N-chunk width (fits one PSUM bank)
NCH = CSH // NW            # 5 chunks per core
KC = D // 128              # 4 contraction chunks
CT = (CSH + 127) // 128    # 20 class tiles for normalize/transpose
S = 30.0
M_MARGIN = 0.5

F32 = mybir.dt.float32
BF16 = mybir.dt.bfloat16

_GRAPH = None


def build_graph():
    nc = bacc.Bacc()
    f_ext = nc.declare_dram_parameter("f", [B, D], F32, isOutput=False)
    wsh_ext = nc.declare_dram_parameter("wsh", [CSH, D], F32, isOutput=False)
    w_ext = nc.declare_dram_parameter("w", [C, D], F32, isOutput=False)
    fsh_ext = nc.declare_dram_parameter("fsh", [BSH, D], F32, isOutput=False)
    pidx_ext = nc.declare_dram_parameter("pidx", [128, 16], mybir.dt.int32, isOutput=False)
    sexp_ext = nc.declare_dram_parameter("sexp", [128, NBLK * NCH], F32, isOutput=True)
    pdot_ext = nc.declare_dram_parameter("pdot", [128, 16], F32, isOutput=True)
    rf_ext = nc.declare_dram_parameter("rf30", [128, NBLK], F32, isOutput=True)
    rw_ext = nc.declare_dram_parameter("rwrec", [128, CT], F32, isOutput=True)

    mult = mybir.AluOpType.mult
    AF = mybir.ActivationFunctionType

    with ExitStack() as ctx:
        tc = ctx.enter_context(tile.TileContext(nc))
        const = ctx.enter_context(tc.tile_pool(name="const", bufs=1))
        resident = ctx.enter_context(tc.tile_pool(name="resident", bufs=1))
        fstage = ctx.enter_context(tc.tile_pool(name="fstage", bufs=3))
        wstage = ctx.enter_context(tc.tile_pool(name="wstage", bufs=3))
        wbfp = ctx.enter_context(tc.tile_pool(name="wbfp", bufs=3))
        scrp = ctx.enter_context(tc.tile_pool(name="scrp", bufs=2))
        esp = ctx.enter_context(tc.tile_pool(name="esp", bufs=3))
        ptr_pool = ctx.enter_context(tc.tile_pool(name="ptr", bufs=2, space="PSUM"))
        pmm_pool = ctx.enter_context(tc.tile_pool(name="pmm", bufs=6, space="PSUM"))

        id_bf = const.tile([128, 128], BF16)
        id_f32 = const.tile([128, 128], F32)
        make_identity(nc, id_bf[:])
        make_identity(nc, id_f32[:])
        zbias = const.tile([128, 1], F32)
        nc.vector.memset(zbias[:], 0.0)
        nbias = const.tile([128, 1], F32)
        nc.vector.memset(nbias[:], -S)

        # resident tensors
        wT = resident.tile([128, KC, CSH], BF16)     # normalized w, transposed
        fT = resident.tile([128, KC, B], BF16)       # raw f, transposed
        G = resident.tile([128, 16, D], F32)         # gathered positive w rows
        fsh_t = resident.tile([128, 2, D], F32)      # this core's f rows (raw)
        sexp_t = resident.tile([128, NBLK * NCH], F32)
        pdot_t = resident.tile([128, 16], F32)
        ss_f = resident.tile([128, NBLK], F32)
        tmp_f = resident.tile([128, NBLK], F32)
        rf30 = resident.tile([128, NBLK], F32)
        ss_w = resident.tile([128, CT], F32)
        tmp_w = resident.tile([128, CT], F32)
        rw_rec = resident.tile([128, CT], F32)
        pidx_t = resident.tile([128, 16], mybir.dt.int32)

        # ---- positives gather (early: overlaps with everything) ----
        nc.sync.dma_start(pidx_t[:], pidx_ext[:, :])
        nc.sync.dma_start(fsh_t[:, 0, :], fsh_ext[0:128, :])
        nc.sync.dma_start(fsh_t[:, 1, :], fsh_ext[128:256, :])
        for s in range(16):
            nc.gpsimd.indirect_dma_start(
                out=G[:, s, :],
                out_offset=None,
                in_=w_ext[:, :],
                in_offset=bass.IndirectOffsetOnAxis(ap=pidx_t[:, s : s + 1], axis=0),
            )

        # ---- w path: row norms, scale to unit rows (bf16), transpose ----
        nc.vector.memset(rw_rec[:], 0.0)  # tail tile covers <128 partitions
        nc.vector.memset(sexp_t[:], 0.0)  # strip-exp fills col m*NCH only
        for ct in range(CT):
            r0 = ct * 128
            rows = min(128, CSH - r0)
            wt = wstage.tile([128, D], F32, tag="w")
            nc.sync.dma_start(wt[:rows, :], wsh_ext[r0 : r0 + rows, :])
            sc = scrp.tile([128, D], F32, tag="scr")
            nc.scalar.activation(
                sc[:rows, :], wt[:rows, :], AF.Square,
                bias=zbias[:rows, :], scale=1.0,
                accum_out=ss_w[:rows, ct : ct + 1],
            )
            nc.scalar.activation(
                tmp_w[:rows, ct : ct + 1], ss_w[:rows, ct : ct + 1],
                AF.Sqrt, bias=zbias[:rows, :], scale=1.0,
            )
            nc.vector.reciprocal(
                rw_rec[:rows, ct : ct + 1], tmp_w[:rows, ct : ct + 1]
            )
            wbf = wbfp.tile([128, D], BF16, tag="wbf")
            nc.vector.tensor_scalar_mul(
                wbf[:rows, :], wt[:rows, :], rw_rec[:rows, ct : ct + 1]
            )
            pt = ptr_pool.tile([128, KC, 128], BF16, tag="ptr")
            for k in range(KC):
                nc.tensor.transpose(
                    pt[:, k, :rows], wbf[:rows, k * 128 : (k + 1) * 128],
                    id_bf[:rows, :rows],
                )
            nc.vector.tensor_copy(
                out=wT[:, :, r0 : r0 + rows], in_=pt[:, :, :rows]
            )
        nc.sync.dma_start(rw_ext[:, :], rw_rec[:])

        # ---- f path: row norms (for ACT scale), raw transpose ----
        for m in range(NBLK):
            ft = fstage.tile([128, D], F32, tag="f")
            nc.sync.dma_start(ft[:], f_ext[m * 128 : (m + 1) * 128, :])
            sc = scrp.tile([128, D], F32, tag="scr")
            nc.scalar.activation(
                sc[:], ft[:], AF.Square,
                bias=zbias[:], scale=1.0,
                accum_out=ss_f[:, m : m + 1],
            )
            pt = ptr_pool.tile([128, KC, 128], F32, tag="ptr")
            for k in range(KC):
                nc.tensor.transpose(
                    pt[:, k, :], ft[:, k * 128 : (k + 1) * 128], id_f32[:]
                )
            nc.vector.tensor_copy(
                out=fT[:, :, m * 128 : (m + 1) * 128], in_=pt[:]
            )
        # rf30 = 30 / |f|  (sqrt(ss/900) then reciprocal)
        nc.scalar.activation(
            tmp_f[:], ss_f[:], AF.Sqrt, bias=zbias[:], scale=1.0 / (S * S)
        )
        nc.vector.reciprocal(rf30[:], tmp_f[:])
        nc.sync.dma_start(rf_ext[:, :], rf30[:])

        # ---- main loop: dots -> exp(30*cos - 30) -> per-row accumulate ----
        for m in range(NBLK):
            ps = [
                pmm_pool.tile([128, NW], F32, tag="mm", name=f"mm_{m}_{n}")
                for n in range(NCH)
            ]
            for k in range(KC):
                for n in range(NCH):
                    nc.tensor.matmul(
                        ps[n][:],
                        fT[:, k, m * 128 : (m + 1) * 128],
                        wT[:, k, n * NW : (n + 1) * NW],
                        start=(k == 0),
                        stop=(k == KC - 1),
                    )
            strip = esp.tile([128, NCH * NW], F32, tag="es", name=f"st{m}")
            for n in range(NCH):
                nc.any.tensor_copy(
                    out=strip[:, n * NW : (n + 1) * NW], in_=ps[n][:]
                )
            edump = esp.tile([128, NCH * NW], BF16, tag="ed", name=f"ed{m}")
            nc.scalar.activation(
                edump[:], strip[:], AF.Exp,
                bias=nbias[:], scale=rf30[:, m : m + 1],
                accum_out=sexp_t[:, m * NCH : m * NCH + 1],
            )
        nc.sync.dma_start(sexp_ext[:, :], sexp_t[:])

        # ---- positive dots: pdot[p, j+8h] = f[h*128+p] . G[p, j+8h] ----
        for h in range(2):
            for j in range(LMAX):
                s = j + LMAX * h
                sc = scrp.tile([128, D], F32, tag="scr")
                nc.vector.scalar_tensor_tensor(
                    out=sc[:], in0=G[:, s, :], scalar=1.0, in1=fsh_t[:, h, :],
                    op0=mult, op1=mult,
                    accum_out=pdot_t[:, s : s + 1],
                )
        nc.sync.dma_start(pdot_ext[:, :], pdot_t[:])

    nc.finalize()
    return nc


def _get_graph():
    global _GRAPH
    if _GRAPH is None:
        _GRAPH = build_graph()
    return _GRAPH


def make_in_maps(f, lab_word2vec, lab_pinds):
    f = np.ascontiguousarray(np.asarray(f, dtype=np.float32))
    w = np.ascontiguousarray(np.asarray(lab_word2vec, dtype=np.float32))
    pinds = np.asarray(lab_pinds, dtype=np.int64)
    in_maps = []
    for i in range(NCORES):
        # slot s = j + 8h at partition p  <-  lab_pinds[i*256 + h*128 + p, j]
        pidx = np.zeros((128, 16), dtype=np.int32)
        for h in range(2):
            for j in range(LMAX):
                pidx[:, j + LMAX * h] = pinds[
                    i * BSH + h * 128 : i * BSH + h * 128 + 128, j
                ]
        in_maps.append(
            {
                "f": f,
                "wsh": np.ascontiguousarray(w[i * CSH : (i + 1) * CSH]),
                "w": w,
                "fsh": np.ascontiguousarray(f[i * BSH : (i + 1) * BSH]),
                "pidx": pidx,
            }
        )
    return in_maps


def combine(outs, lab_pinds, lengths):
    """outs: list of 8 dicts with sexp/pdot/rf30/rwrec. Returns float32 loss."""
    pinds = np.asarray(lab_pinds, dtype=np.int64)
    lens = np.asarray(lengths, dtype=np.int64)

    # S_shift[b] = sum_c exp(30 cos - 30)
    s_shift = np.zeros(B, dtype=np.float64)
    for i in range(NCORES):
        se = outs[i]["sexp"].astype(np.float64)  # [128, NBLK*NCH]
        per_block = se.reshape(128, NBLK, NCH).sum(axis=2)  # [128, NBLK]
        s_shift += per_block.T.reshape(B)  # b = m*128 + p

    rf = outs[0]["rf30"].astype(np.float64).T.reshape(B) / S  # 1/|f_b|

    rw = np.zeros(C, dtype=np.float64)
    for i in range(NCORES):
        rr = outs[i]["rwrec"].astype(np.float64)  # [128, CT]
        for ct in range(CT):
            r0 = ct * 128
            rows = min(128, CSH - r0)
            rw[i * CSH + r0 : i * CSH + r0 + rows] = rr[:rows, ct]

    # positive raw dots -> [B, LMAX]
    pdot = np.zeros((B, LMAX), dtype=np.float64)
    for i in range(NCORES):
        pd = outs[i]["pdot"].astype(np.float64)  # [128, 16]
        for h in range(2):
            for j in range(LMAX):
                pdot[i * BSH + h * 128 : i * BSH + h * 128 + 128, j] = pd[
                    :, j + LMAX * h
                ]

    cos = pdot * rf[:, None] * rw[pinds]  # [B, LMAX]
    cos_m, sin_m = math.cos(M_MARGIN), math.sin(M_MARGIN)
    th = math.cos(math.pi - M_MARGIN)
    mm = math.sin(math.pi - M_MARGIN) * M_MARGIN
    sine = np.sqrt(np.clip(1.0 - cos * cos, 0.0, 1.0))
    phi = cos * cos_m - sine * sin_m
    phi = np.where(cos > th, phi, cos - mm)

    mask = (np.arange(LMAX)[None, :] < lens[:, None]).astype(np.float64)
    corr = (mask * (np.exp(S * phi - S) - np.exp(S * cos - S))).sum(axis=1)
    z = S + np.log(s_shift + corr)  # logsumexp of outputs, [B]
    pos_sum = (mask * (S * phi)).sum(axis=1)
    L = lens.astype(np.float64)
    per_sample = (L * z - pos_sum) / (L * L)
    return np.float32(per_sample.mean())


def kernel(f, labels, lab_word2vec, lab_pinds, lengths):
    nc = _get_graph()
    in_maps = make_in_maps(f, lab_word2vec, lab_pinds)
    res = run_bass_kernel_spmd(nc, in_maps, core_ids=list(range(NCORES)))
    return combine(res.results, lab_pinds, lengths)


# revision 33
# speedup vs baseline: 1.6538x; 1.1111x over previous
"""ArcFace combined-margin loss kernel for 8 TRN2 NeuronCores.

Strategy
--------
reference: cos = (f @ w.T) / (|f||w|); phi = arcface(cos);
outputs = s*(labels*phi + (1-labels)*cos); loss = mean over rows of
-(sum of log_softmax(outputs) at lab_pinds, masked) / L^2.

labels is the multi-hot of (lab_pinds, lengths), so outputs differs from
s*cos only at <=8 entries/row.  Device work is therefore:
  1. C-sharded (2500 classes/core) dense part: each core computes, for all
     2048 rows, partial sums  sexp[b] = sum_c exp(30*cos[b,c] - 30)  over its
     class shard (bf16 matmul of unit-normalized w rows against raw f rows,
     transposed on the TensorEngine; ACT exp with per-partition scale
     30/|f_b| and free-dim accumulate, reading PSUM directly).
  2. B-sharded (256 rows/core) positive part: indirect-DMA gather of the
     2048 w rows addressed by lab_pinds, raw fp32 dots with f rows on DVE.
  3. Per-row norm reciprocals (30/|f_b| and 1/|w_c|) as side outputs.
Host (numpy, float64) combines the tiny per-core partials: assembles
cos at positives, applies the arcface margin, corrects the denominator
(exp(30*phi)-exp(30*cos) at positives), logsumexp, masked ragged CE, mean.
No collectives are needed (the only cross-core reduction is over [2048]
scalars, done on host during unsharding).
"""

import math
import sys

import numpy as np

for _p in ("/opt/trn_rl_repo",):
    if _p not in sys.path:
        sys.path.append(_p)

import concourse.bass as bass
import concourse.bacc as bacc
import concourse.mybir as mybir
import concourse.tile as tile
from concourse.bass_utils import run_bass_kernel_spmd
from concourse.masks import make_identity
from contextlib import ExitStack

B, C, D, LMAX = 2048, 20000, 512, 8
NCORES = 8
CSH = C // NCORES          # 2500 classes per core
BSH = B // NCORES          # 256 rows per core (positives shard)
NBLK = B // 128            # 16 row blocks
NW = 500                   # matmul N-chunk width (fits one PSUM bank)
NCH = CSH // NW            # 5 chunks per core
KC = D // 128              # 4 contraction chunks
CT = (CSH + 127) // 128    # 20 class tiles for normalize/transpose
S = 30.0
M_MARGIN = 0.5

F32 = mybir.dt.float32
BF16 = mybir.dt.bfloat16
FP8 = mybir.dt.float8e4
F8S = 16.0                 # fp8 pre-scale per operand (dots carry 256x)

_GRAPH = None


def build_graph():
    nc = bacc.Bacc()
    f_ext = nc.declare_dram_parameter("f", [B, D], F32, isOutput=False)
    wsh_ext = nc.declare_dram_parameter("wsh", [CSH, D], F32, isOutput=False)
    w_ext = nc.declare_dram_parameter("w", [C, D], F32, isOutput=False)
    fsh_ext = nc.declare_dram_parameter("fsh", [BSH, D], F32, isOutput=False)
    pidx_ext = nc.declare_dram_parameter("pidx", [128, 16], mybir.dt.int32, isOutput=False)
    sexp_ext = nc.declare_dram_parameter("sexp", [128, NBLK * NCH], F32, isOutput=True)
    pdot_ext = nc.declare_dram_parameter("pdot", [128, 16], F32, isOutput=True)
    rf_ext = nc.declare_dram_parameter("rf30", [128, NBLK], F32, isOutput=True)
    rw_ext = nc.declare_dram_parameter("rwrec", [128, CT], F32, isOutput=True)

    mult = mybir.AluOpType.mult
    AF = mybir.ActivationFunctionType

    with ExitStack() as ctx:
        tc = ctx.enter_context(tile.TileContext(nc))
        const = ctx.enter_context(tc.tile_pool(name="const", bufs=1))
        resident = ctx.enter_context(tc.tile_pool(name="resident", bufs=1))
        fstage = ctx.enter_context(tc.tile_pool(name="fstage", bufs=3))
        wstage = ctx.enter_context(tc.tile_pool(name="wstage", bufs=3))
        wbfp = ctx.enter_context(tc.tile_pool(name="wbfp", bufs=3))
        scrp = ctx.enter_context(tc.tile_pool(name="scrp", bufs=2))
        esp = ctx.enter_context(tc.tile_pool(name="esp", bufs=3))
        ptr_pool = ctx.enter_context(tc.tile_pool(name="ptr", bufs=2, space="PSUM"))
        pmm_pool = ctx.enter_context(tc.tile_pool(name="pmm", bufs=6, space="PSUM"))

        id_bf = const.tile([128, 128], BF16)
        id_f32 = const.tile([128, 128], F32)
        make_identity(nc, id_bf[:])
        make_identity(nc, id_f32[:])
        zbias = const.tile([128, 1], F32)
        nc.vector.memset(zbias[:], 0.0)
        nbias = const.tile([128, 1], F32)
        nc.vector.memset(nbias[:], -S)

        # resident tensors
        wT = resident.tile([128, KC, CSH], FP8)      # normalized w, transposed
        fT = resident.tile([128, KC, B], FP8)        # raw f, transposed
        G = resident.tile([128, 16, D], F32)         # gathered positive w rows
        fsh_t = resident.tile([128, 2, D], F32)      # this core's f rows (raw)
        sexp_t = resident.tile([128, NBLK * NCH], F32)
        pdot_t = resident.tile([128, 16], F32)
        ss_f = resident.tile([128, NBLK], F32)
        tmp_f = resident.tile([128, NBLK], F32)
        rf30 = resident.tile([128, NBLK], F32)
        rf30s = resident.tile([128, NBLK], F32)
        ss_w = resident.tile([128, CT], F32)
        tmp_w = resident.tile([128, CT], F32)
        rw_rec = resident.tile([128, CT], F32)
        pidx_t = resident.tile([128, 16], mybir.dt.int32)

        # ---- positives gather (early: overlaps with everything) ----
        nc.sync.dma_start(pidx_t[:], pidx_ext[:, :])
        nc.sync.dma_start(fsh_t[:, 0, :], fsh_ext[0:128, :])
        nc.sync.dma_start(fsh_t[:, 1, :], fsh_ext[128:256, :])
        for s in range(16):
            nc.gpsimd.indirect_dma_start(
                out=G[:, s, :],
                out_offset=None,
                in_=w_ext[:, :],
                in_offset=bass.IndirectOffsetOnAxis(ap=pidx_t[:, s : s + 1], axis=0),
            )

        # ---- w path: row norms, scale to unit rows (bf16), transpose ----
        nc.vector.memset(rw_rec[:], 0.0)  # tail tile covers <128 partitions
        nc.vector.memset(sexp_t[:], 0.0)  # strip-exp fills col m*NCH only
        for ct in range(CT):
            r0 = ct * 128
            rows = min(128, CSH - r0)
            wt = wstage.tile([128, D], F32, tag="w")
            nc.sync.dma_start(wt[:rows, :], wsh_ext[r0 : r0 + rows, :])
            sc = scrp.tile([128, D], F32, tag="scr")
            nc.scalar.activation(
                sc[:rows, :], wt[:rows, :], AF.Square,
                bias=zbias[:rows, :], scale=1.0,
                accum_out=ss_w[:rows, ct : ct + 1],
            )
            nc.scalar.activation(
                tmp_w[:rows, ct : ct + 1], ss_w[:rows, ct : ct + 1],
                AF.Sqrt, bias=zbias[:rows, :], scale=1.0,
            )
            nc.vector.reciprocal(
                rw_rec[:rows, ct : ct + 1], tmp_w[:rows, ct : ct + 1]
            )
            wbf = wbfp.tile([128, D], BF16, tag="wbf")
            nc.vector.tensor_scalar_mul(
                wbf[:rows, :], wt[:rows, :], rw_rec[:rows, ct : ct + 1]
            )
            pt = ptr_pool.tile([128, KC, 128], BF16, tag="ptr")
            for k in range(KC):
                nc.tensor.transpose(
                    pt[:, k, :rows], wbf[:rows, k * 128 : (k + 1) * 128],
                    id_bf[:rows, :rows],
                )
            nc.vector.tensor_scalar_mul(
                wT[:, :, r0 : r0 + rows], pt[:, :, :rows], F8S
            )
        nc.sync.dma_start(rw_ext[:, :], rw_rec[:])

        # ---- f path: row norms (for ACT scale), raw transpose ----
        for m in range(NBLK):
            ft = fstage.tile([128, D], F32, tag="f")
            nc.sync.dma_start(ft[:], f_ext[m * 128 : (m + 1) * 128, :])
            sc = scrp.tile([128, D], F32, tag="scr")
            nc.scalar.activation(
                sc[:], ft[:], AF.Square,
                bias=zbias[:], scale=1.0,
                accum_out=ss_f[:, m : m + 1],
            )
            pt = ptr_pool.tile([128, KC, 128], F32, tag="ptr")
            for k in range(KC):
                nc.tensor.transpose(
                    pt[:, k, :], ft[:, k * 128 : (k + 1) * 128], id_f32[:]
                )
            nc.vector.tensor_scalar_mul(
                fT[:, :, m * 128 : (m + 1) * 128], pt[:], F8S
            )
        # rf30 = 30 / |f|  (sqrt(ss/900) then reciprocal)
        nc.scalar.activation(
            tmp_f[:], ss_f[:], AF.Sqrt, bias=zbias[:], scale=1.0 / (S * S)
        )
        nc.vector.reciprocal(rf30[:], tmp_f[:])
        nc.vector.tensor_scalar_mul(rf30s[:], rf30[:], 1.0 / (F8S * F8S))
        nc.sync.dma_start(rf_ext[:, :], rf30[:])

        # ---- main loop: dots -> exp(30*cos - 30) -> per-row accumulate ----
        for m in range(NBLK):
            ps = [
                pmm_pool.tile([128, NW], F32, tag="mm", name=f"mm_{m}_{n}")
                for n in range(NCH)
            ]
            for k2 in range(KC // 2):
                for n in range(NCH):
                    nc.tensor.matmul(
                        ps[n][:],
                        fT[:, 2 * k2 : 2 * k2 + 2, m * 128 : (m + 1) * 128],
                        wT[:, 2 * k2 : 2 * k2 + 2, n * NW : (n + 1) * NW],
                        start=(k2 == 0),
                        stop=(k2 == KC // 2 - 1),
                        perf_mode=mybir.MatmulPerfMode.DoubleRow,
                    )
            strip = esp.tile([128, NCH * NW], F32, tag="es", name=f"st{m}")
            for n in range(NCH):
                nc.any.tensor_copy(
                    out=strip[:, n * NW : (n + 1) * NW], in_=ps[n][:]
                )
            edump = esp.tile([128, NCH * NW], BF16, tag="ed", name=f"ed{m}")
            nc.scalar.activation(
                edump[:], strip[:], AF.Exp,
                bias=nbias[:], scale=rf30s[:, m : m + 1],
                accum_out=sexp_t[:, m * NCH : m * NCH + 1],
            )
        nc.sync.dma_start(sexp_ext[:, :], sexp_t[:])

        # ---- positive dots: pdot[p, j+8h] = f[h*128+p] . G[p, j+8h] ----
        for h in range(2):
            for j in range(LMAX):
                s = j + LMAX * h
                sc = scrp.tile([128, D], F32, tag="scr")
                nc.vector.scalar_tensor_tensor(
                    out=sc[:], in0=G[:, s, :], scalar=1.0, in1=fsh_t[:, h, :],
                    op0=mult, op1=mult,
                    accum_out=pdot_t[:, s : s + 1],
                )
        nc.sync.dma_start(pdot_ext[:, :], pdot_t[:])

    nc.finalize()
    return nc


def _get_graph():
    global _GRAPH
    if _GRAPH is None:
        _GRAPH = build_graph()
    return _GRAPH


def make_in_maps(f, lab_word2vec, lab_pinds):
    f = np.ascontiguousarray(np.asarray(f, dtype=np.float32))
    w = np.ascontiguousarray(np.asarray(lab_word2vec, dtype=np.float32))
    pinds = np.asarray(lab_pinds, dtype=np.int64)
    in_maps = []
    for i in range(NCORES):
        # slot s = j + 8h at partition p  <-  lab_pinds[i*256 + h*128 + p, j]
        pidx = np.zeros((128, 16), dtype=np.int32)
        for h in range(2):
            for j in range(LMAX):
                pidx[:, j + LMAX * h] = pinds[
                    i * BSH + h * 128 : i * BSH + h * 128 + 128, j
                ]
        in_maps.append(
            {
                "f": f,
                "wsh": np.ascontiguousarray(w[i * CSH : (i + 1) * CSH]),
                "w": w,
                "fsh": np.ascontiguousarray(f[i * BSH : (i + 1) * BSH]),
                "pidx": pidx,
            }
        )
    return in_maps


def combine(outs, lab_pinds, lengths):
    """outs: list of 8 dicts with sexp/pdot/rf30/rwrec. Returns float32 loss."""
    pinds = np.asarray(lab_pinds, dtype=np.int64)
    lens = np.asarray(lengths, dtype=np.int64)

    # S_shift[b] = sum_c exp(30 cos - 30)
    s_shift = np.zeros(B, dtype=np.float64)
    for i in range(NCORES):
        se = outs[i]["sexp"].astype(np.float64)  # [128, NBLK*NCH]
        per_block = se.reshape(128, NBLK, NCH).sum(axis=2)  # [128, NBLK]
        s_shift += per_block.T.reshape(B)  # b = m*128 + p

    rf = outs[0]["rf30"].astype(np.float64).T.reshape(B) / S  # 1/|f_b|

    rw = np.zeros(C, dtype=np.float64)
    for i in range(NCORES):
        rr = outs[i]["rwrec"].astype(np.float64)  # [128, CT]
        for ct in range(CT):
            r0 = ct * 128
            rows = min(128, CSH - r0)
            rw[i * CSH + r0 : i * CSH + r0 + rows] = rr[:rows, ct]

    # positive raw dots -> [B, LMAX]
    pdot = np.zeros((B, LMAX), dtype=np.float64)
    for i in range(NCORES):
        pd = outs[i]["pdot"].astype(np.float64)  # [128, 16]
        for h in range(2):
            for j in range(LMAX):
                pdot[i * BSH + h * 128 : i * BSH + h * 128 + 128, j] = pd[
                    :, j + LMAX * h
                ]

    cos = pdot * rf[:, None] * rw[pinds]  # [B, LMAX]
    cos_m, sin_m = math.cos(M_MARGIN), math.sin(M_MARGIN)
    th = math.cos(math.pi - M_MARGIN)
    mm = math.sin(math.pi - M_MARGIN) * M_MARGIN
    sine = np.sqrt(np.clip(1.0 - cos * cos, 0.0, 1.0))
    phi = cos * cos_m - sine * sin_m
    phi = np.where(cos > th, phi, cos - mm)

    mask = (np.arange(LMAX)[None, :] < lens[:, None]).astype(np.float64)
    corr = (mask * (np.exp(S * phi - S) - np.exp(S * cos - S))).sum(axis=1)
    z = S + np.log(s_shift + corr)  # logsumexp of outputs, [B]
    pos_sum = (mask * (S * phi)).sum(axis=1)
    L = lens.astype(np.float64)
    per_sample = (L * z - pos_sum) / (L * L)
    return np.float32(per_sample.mean())


def kernel(f, labels, lab_word2vec, lab_pinds, lengths):
    nc = _get_graph()
    in_maps = make_in_maps(f, lab_word2vec, lab_pinds)
    res = run_bass_kernel_spmd(nc, in_maps, core_ids=list(range(NCORES)))
    return combine(res.results, lab_pinds, lengths)


# revision 34
# speedup vs baseline: 1.7102x; 1.0341x over previous
"""ArcFace combined-margin loss kernel for 8 TRN2 NeuronCores.

Strategy
--------
reference: cos = (f @ w.T) / (|f||w|); phi = arcface(cos);
outputs = s*(labels*phi + (1-labels)*cos); loss = mean over rows of
-(sum of log_softmax(outputs) at lab_pinds, masked) / L^2.

labels is the multi-hot of (lab_pinds, lengths), so outputs differs from
s*cos only at <=8 entries/row.  Device work is therefore:
  1. C-sharded (2500 classes/core) dense part: each core computes, for all
     2048 rows, partial sums  sexp[b] = sum_c exp(30*cos[b,c] - 30)  over its
     class shard (bf16 matmul of unit-normalized w rows against raw f rows,
     transposed on the TensorEngine; ACT exp with per-partition scale
     30/|f_b| and free-dim accumulate, reading PSUM directly).
  2. B-sharded (256 rows/core) positive part: indirect-DMA gather of the
     2048 w rows addressed by lab_pinds, raw fp32 dots with f rows on DVE.
  3. Per-row norm reciprocals (30/|f_b| and 1/|w_c|) as side outputs.
Host (numpy, float64) combines the tiny per-core partials: assembles
cos at positives, applies the arcface margin, corrects the denominator
(exp(30*phi)-exp(30*cos) at positives), logsumexp, masked ragged CE, mean.
No collectives are needed (the only cross-core reduction is over [2048]
scalars, done on host during unsharding).
"""

import math
import sys

import numpy as np

for _p in ("/opt/trn_rl_repo",):
    if _p not in sys.path:
        sys.path.append(_p)

import concourse.bass as bass
import concourse.bacc as bacc
import concourse.mybir as mybir
import concourse.tile as tile
from concourse.bass_utils import run_bass_kernel_spmd
from concourse.masks import make_identity
from contextlib import ExitStack

B, C, D, LMAX = 2048, 20000, 512, 8
NCORES = 8
CSH = C // NCORES          # 2500 real classes per core
CSHP = 2560                # padded to 5*512 (bank-aligned chunks)
BSH = B // NCORES          # 256 rows per core (positives shard)
NBLK = B // 128            # 16 row blocks
NW = 512                   # matmul N-chunk width (exactly one PSUM bank)
NCH = CSHP // NW           # 5 chunks per core
KC = D // 128              # 4 contraction chunks
CT = CSHP // 128           # 20 class tiles (all full)
S = 30.0
M_MARGIN = 0.5

F32 = mybir.dt.float32
BF16 = mybir.dt.bfloat16
FP8 = mybir.dt.float8e4
F8S = 16.0                 # fp8 pre-scale per operand (dots carry 256x)

_GRAPH = None


def build_graph():
    nc = bacc.Bacc()
    f_ext = nc.declare_dram_parameter("f", [B, D], F32, isOutput=False)
    wsh_ext = nc.declare_dram_parameter("wsh", [CSHP, D], F32, isOutput=False)
    w_ext = nc.declare_dram_parameter("w", [C, D], F32, isOutput=False)
    fsh_ext = nc.declare_dram_parameter("fsh", [BSH, D], F32, isOutput=False)
    pidx_ext = nc.declare_dram_parameter("pidx", [128, 16], mybir.dt.int32, isOutput=False)
    sexp_ext = nc.declare_dram_parameter("sexp", [128, NBLK * NCH], F32, isOutput=True)
    pdot_ext = nc.declare_dram_parameter("pdot", [128, 16], F32, isOutput=True)
    rf_ext = nc.declare_dram_parameter("rf30", [128, NBLK], F32, isOutput=True)
    rw_ext = nc.declare_dram_parameter("rwrec", [128, CT], F32, isOutput=True)

    mult = mybir.AluOpType.mult
    AF = mybir.ActivationFunctionType

    with ExitStack() as ctx:
        tc = ctx.enter_context(tile.TileContext(nc))
        const = ctx.enter_context(tc.tile_pool(name="const", bufs=1))
        resident = ctx.enter_context(tc.tile_pool(name="resident", bufs=1))
        fstage = ctx.enter_context(tc.tile_pool(name="fstage", bufs=3))
        wstage = ctx.enter_context(tc.tile_pool(name="wstage", bufs=3))
        wbfp = ctx.enter_context(tc.tile_pool(name="wbfp", bufs=3))
        scrp = ctx.enter_context(tc.tile_pool(name="scrp", bufs=2))
        esp = ctx.enter_context(tc.tile_pool(name="esp", bufs=3))
        ptr_pool = ctx.enter_context(tc.tile_pool(name="ptr", bufs=1, space="PSUM"))
        pmmA = ctx.enter_context(tc.tile_pool(name="pmmA", bufs=2, space="PSUM"))
        pmmB = ctx.enter_context(tc.tile_pool(name="pmmB", bufs=1, space="PSUM"))
        pmmC = ctx.enter_context(tc.tile_pool(name="pmmC", bufs=1, space="PSUM"))

        id_bf = const.tile([128, 128], BF16)
        id_f32 = const.tile([128, 128], F32)
        make_identity(nc, id_bf[:])
        make_identity(nc, id_f32[:])
        zbias = const.tile([128, 1], F32)
        nc.vector.memset(zbias[:], 0.0)
        nbias = const.tile([128, 1], F32)
        nc.vector.memset(nbias[:], -S)

        # resident tensors
        wT = resident.tile([128, KC, CSHP], FP8)      # normalized w, transposed
        fT = resident.tile([128, KC, B], FP8)        # raw f, transposed
        G = resident.tile([128, 16, D], F32)         # gathered positive w rows
        fsh_t = resident.tile([128, 2, D], F32)      # this core's f rows (raw)
        sexp_t = resident.tile([128, NBLK * NCH], F32)
        pdot_t = resident.tile([128, 16], F32)
        ss_f = resident.tile([128, NBLK], F32)
        tmp_f = resident.tile([128, NBLK], F32)
        rf30 = resident.tile([128, NBLK], F32)
        rf30s = resident.tile([128, NBLK], F32)
        ss_w = resident.tile([128, CT], F32)
        tmp_w = resident.tile([128, CT], F32)
        rw_rec = resident.tile([128, CT], F32)
        pidx_t = resident.tile([128, 16], mybir.dt.int32)

        # ---- positives gather (early: overlaps with everything) ----
        nc.sync.dma_start(pidx_t[:], pidx_ext[:, :])
        nc.sync.dma_start(fsh_t[:, 0, :], fsh_ext[0:128, :])
        nc.sync.dma_start(fsh_t[:, 1, :], fsh_ext[128:256, :])
        for s in range(16):
            nc.gpsimd.indirect_dma_start(
                out=G[:, s, :],
                out_offset=None,
                in_=w_ext[:, :],
                in_offset=bass.IndirectOffsetOnAxis(ap=pidx_t[:, s : s + 1], axis=0),
            )

        # ---- w path: row norms, scale to unit rows (bf16), transpose ----
        nc.vector.memset(rw_rec[:], 0.0)  # tail tile covers <128 partitions
        nc.vector.memset(sexp_t[:], 0.0)  # strip-exp fills col m*NCH only
        for ct in range(CT):
            r0 = ct * 128
            wt = wstage.tile([128, D], F32, tag="w")
            nc.sync.dma_start(wt[:, :], wsh_ext[r0 : r0 + 128, :])
            sc = scrp.tile([128, D], F32, tag="scr")
            nc.scalar.activation(
                sc[:], wt[:], AF.Square,
                bias=zbias[:], scale=1.0,
                accum_out=ss_w[:, ct : ct + 1],
            )
            # zero pad rows: keep sqrt/recip finite (wT pad cols end up 0)
            nc.vector.tensor_scalar_max(
                ss_w[:, ct : ct + 1], ss_w[:, ct : ct + 1], 1e-12
            )
            nc.scalar.activation(
                tmp_w[:, ct : ct + 1], ss_w[:, ct : ct + 1],
                AF.Sqrt, bias=zbias[:], scale=1.0,
            )
            nc.vector.reciprocal(
                rw_rec[:, ct : ct + 1], tmp_w[:, ct : ct + 1]
            )
            wbf = wbfp.tile([128, D], BF16, tag="wbf")
            nc.vector.tensor_scalar_mul(
                wbf[:, :], wt[:, :], rw_rec[:, ct : ct + 1]
            )
            pt = ptr_pool.tile([128, KC, 128], BF16, tag="ptr")
            for k in range(KC):
                nc.tensor.transpose(
                    pt[:, k, :], wbf[:, k * 128 : (k + 1) * 128], id_bf[:]
                )
            nc.vector.tensor_scalar_mul(
                wT[:, :, r0 : r0 + 128], pt[:], F8S
            )
        nc.sync.dma_start(rw_ext[:, :], rw_rec[:])

        # ---- f path: row norms (for ACT scale), raw transpose ----
        for m in range(NBLK):
            ft = fstage.tile([128, D], F32, tag="f")
            nc.sync.dma_start(ft[:], f_ext[m * 128 : (m + 1) * 128, :])
            sc = scrp.tile([128, D], F32, tag="scr")
            nc.scalar.activation(
                sc[:], ft[:], AF.Square,
                bias=zbias[:], scale=1.0,
                accum_out=ss_f[:, m : m + 1],
            )
            pt = ptr_pool.tile([128, KC, 128], F32, tag="ptr")
            for k in range(KC):
                nc.tensor.transpose(
                    pt[:, k, :], ft[:, k * 128 : (k + 1) * 128], id_f32[:]
                )
            nc.vector.tensor_scalar_mul(
                fT[:, :, m * 128 : (m + 1) * 128], pt[:], F8S
            )
        # rf30 = 30 / |f|  (sqrt(ss/900) then reciprocal)
        nc.scalar.activation(
            tmp_f[:], ss_f[:], AF.Sqrt, bias=zbias[:], scale=1.0 / (S * S)
        )
        nc.vector.reciprocal(rf30[:], tmp_f[:])
        nc.vector.tensor_scalar_mul(rf30s[:], rf30[:], 1.0 / (F8S * F8S))
        nc.sync.dma_start(rf_ext[:, :], rf30[:])


        # ---- positive dots: pdot[p, j+8h] = f[h*128+p] . G[p, j+8h] ----
        for h in range(2):
            for j in range(LMAX):
                s = j + LMAX * h
                sc = scrp.tile([128, D], F32, tag="scr")
                nc.vector.scalar_tensor_tensor(
                    out=sc[:], in0=G[:, s, :], scalar=1.0, in1=fsh_t[:, h, :],
                    op0=mult, op1=mult,
                    accum_out=pdot_t[:, s : s + 1],
                )
        nc.sync.dma_start(pdot_ext[:, :], pdot_t[:])

        # ---- main loop: dots -> exp(30*cos - 30) -> per-row accumulate ----
        for m in range(NBLK):
            pA = pmmA.tile([128, 2, NW], F32, tag="mmA", name=f"mmA_{m}")
            pB = pmmB.tile([128, 2, NW], F32, tag="mmB", name=f"mmB_{m}")
            pC = pmmC.tile([128, NW], F32, tag="mmC", name=f"mmC_{m}")
            ps = [pA[:, 0, :], pA[:, 1, :], pB[:, 0, :], pB[:, 1, :], pC[:]]
            for k2 in range(KC // 2):
                for n in range(NCH):
                    nc.tensor.matmul(
                        ps[n],
                        fT[:, 2 * k2 : 2 * k2 + 2, m * 128 : (m + 1) * 128],
                        wT[:, 2 * k2 : 2 * k2 + 2, n * NW : (n + 1) * NW],
                        start=(k2 == 0),
                        stop=(k2 == KC // 2 - 1),
                        perf_mode=mybir.MatmulPerfMode.DoubleRow,
                    )
            strip = esp.tile([128, NCH * NW], F32, tag="es", name=f"st{m}")
            nc.any.tensor_copy(out=strip[:, 0 : 2 * NW], in_=pA[:])
            nc.any.tensor_copy(out=strip[:, 2 * NW : 4 * NW], in_=pB[:])
            nc.any.tensor_copy(out=strip[:, 4 * NW : 5 * NW], in_=pC[:])
            edump = esp.tile([128, NCH * NW], BF16, tag="ed", name=f"ed{m}")
            nc.scalar.activation(
                edump[:], strip[:], AF.Exp,
                bias=nbias[:], scale=rf30s[:, m : m + 1],
                accum_out=sexp_t[:, m * NCH : m * NCH + 1],
            )
        nc.sync.dma_start(sexp_ext[:, :], sexp_t[:])


    nc.finalize()
    return nc


def _get_graph():
    global _GRAPH
    if _GRAPH is None:
        _GRAPH = build_graph()
    return _GRAPH


def make_in_maps(f, lab_word2vec, lab_pinds):
    f = np.ascontiguousarray(np.asarray(f, dtype=np.float32))
    w = np.ascontiguousarray(np.asarray(lab_word2vec, dtype=np.float32))
    pinds = np.asarray(lab_pinds, dtype=np.int64)
    in_maps = []
    for i in range(NCORES):
        # slot s = j + 8h at partition p  <-  lab_pinds[i*256 + h*128 + p, j]
        pidx = np.zeros((128, 16), dtype=np.int32)
        for h in range(2):
            for j in range(LMAX):
                pidx[:, j + LMAX * h] = pinds[
                    i * BSH + h * 128 : i * BSH + h * 128 + 128, j
                ]
        wsh = np.zeros((CSHP, D), dtype=np.float32)
        wsh[:CSH] = w[i * CSH : (i + 1) * CSH]
        in_maps.append(
            {
                "f": f,
                "wsh": wsh,
                "w": w,
                "fsh": np.ascontiguousarray(f[i * BSH : (i + 1) * BSH]),
                "pidx": pidx,
            }
        )
    return in_maps


def combine(outs, lab_pinds, lengths):
    """outs: list of 8 dicts with sexp/pdot/rf30/rwrec. Returns float32 loss."""
    pinds = np.asarray(lab_pinds, dtype=np.int64)
    lens = np.asarray(lengths, dtype=np.int64)

    # S_shift[b] = sum_c exp(30 cos - 30)
    s_shift = np.zeros(B, dtype=np.float64)
    for i in range(NCORES):
        se = outs[i]["sexp"].astype(np.float64)  # [128, NBLK*NCH]
        per_block = se.reshape(128, NBLK, NCH).sum(axis=2)  # [128, NBLK]
        s_shift += per_block.T.reshape(B)  # b = m*128 + p
    # the 60 zero-pad classes per core contribute exp(-30) each (cos = 0)
    s_shift -= NCORES * (CSHP - CSH) * math.exp(-S)

    rf = outs[0]["rf30"].astype(np.float64).T.reshape(B) / S  # 1/|f_b|

    rw = np.zeros(C, dtype=np.float64)
    for i in range(NCORES):
        rr = outs[i]["rwrec"].astype(np.float64)  # [128, CT]
        rw[i * CSH : (i + 1) * CSH] = rr.T.reshape(CSHP)[:CSH]

    # positive raw dots -> [B, LMAX]
    pdot = np.zeros((B, LMAX), dtype=np.float64)
    for i in range(NCORES):
        pd = outs[i]["pdot"].astype(np.float64)  # [128, 16]
        for h in range(2):
            for j in range(LMAX):
                pdot[i * BSH + h * 128 : i * BSH + h * 128 + 128, j] = pd[
                    :, j + LMAX * h
                ]

    cos = pdot * rf[:, None] * rw[pinds]  # [B, LMAX]
    cos_m, sin_m = math.cos(M_MARGIN), math.sin(M_MARGIN)
    th = math.cos(math.pi - M_MARGIN)
    mm = math.sin(math.pi - M_MARGIN) * M_MARGIN
    sine = np.sqrt(np.clip(1.0 - cos * cos, 0.0, 1.0))
    phi = cos * cos_m - sine * sin_m
    phi = np.where(cos > th, phi, cos - mm)

    mask = (np.arange(LMAX)[None, :] < lens[:, None]).astype(np.float64)
    corr = (mask * (np.exp(S * phi - S) - np.exp(S * cos - S))).sum(axis=1)
    z = S + np.log(s_shift + corr)  # logsumexp of outputs, [B]
    pos_sum = (mask * (S * phi)).sum(axis=1)
    L = lens.astype(np.float64)
    per_sample = (L * z - pos_sum) / (L * L)
    return np.float32(per_sample.mean())


def kernel(f, labels, lab_word2vec, lab_pinds, lengths):
    nc = _get_graph()
    in_maps = make_in_maps(f, lab_word2vec, lab_pinds)
    res = run_bass_kernel_spmd(nc, in_maps, core_ids=list(range(NCORES)))
    return combine(res.results, lab_pinds, lengths)
